# revision 15
# baseline (speedup 1.0000x reference)
"""MultiHeadAttention TRN2 kernel (B=4, S=2048, D=128, H=8) over 8 NeuronCores.

Sharding: core c handles batch b = c//2 and head-group hg = c%2 (4 heads).
Each core computes its 4 heads' attn [4, S, S] and its out rows [1024, 128].

Per-core algorithm:
  - projections qp/kp/vp on PE (fp32), heads padded to 32-partition bases
  - per-head stacked bf16 operand tiles qxh/kxh [128, S] (hi/mid split so a
    single K=128 matmul carries hi*hi + hi*mid + mid*hi ~ near-fp32 logits):
      qxh rows [0:16]=hi [32:48]=hi [64:80]=mid [96]=ones   (rest zero)
      kxh rows [0:16]=hi [32:48]=mid [64:80]=hi [96]=mask*-1e9
  - T-side (per 1024-wide q block): logitsT[k, q] = kx.T @ qx (bf16),
    expT = Exp(logitsT) in [128,1024] ACT ops (fp32r out),
    ctxT[17, 1024] += vhx.T @ expT (fp32r; col 16 of vhx = ones -> row sums)
  - straight side (per q tile): logits [q,k] = qx.T @ kx (mask via row 96),
    exp (ACT) -> attn = exp * recip[q] (DVE) -> DMA out
  - ctx sandwich: ctxT -> PE transpose -> recip -> normalize -> transpose back
  - out-proj: reshape-aware K=128 stacked fp32r matmul with dense_w hi/lo
"""
import sys

if '/opt/trn_rl_repo' not in sys.path:
    sys.path.insert(0, '/opt/trn_rl_repo')

import contextlib

import numpy as np

B, S, D, H = 4, 2048, 128, 8
DP = D // H
HPC = 4
NCORES = 8
NKT = S // 128
QC = 512

_BUILT = {}


def _build_nc():
    import concourse.mybir as mybir
    import concourse.tile as tile
    from concourse import bacc
    from concourse.masks import make_identity

    F32 = mybir.dt.float32
    F32R = mybir.dt.float32r
    BF16 = mybir.dt.bfloat16
    Act = mybir.ActivationFunctionType
    Alu = mybir.AluOpType

    nc = bacc.Bacc("TRN2", target_bir_lowering=False, debug=False)

    QT = nc.dram_tensor("QT", [D, S], F32, kind="ExternalInput")
    KT = nc.dram_tensor("KT", [D, S], F32, kind="ExternalInput")
    VT = nc.dram_tensor("VT", [D, S], F32, kind="ExternalInput")
    WQX = nc.dram_tensor("WQX", [D, 128], F32, kind="ExternalInput")
    WQCOLQ = nc.dram_tensor("WQCOLQ", [128, 1], F32, kind="ExternalInput")
    WQCOLK = nc.dram_tensor("WQCOLK", [128, 1], F32, kind="ExternalInput")
    WQROWV = nc.dram_tensor("WQROWV", [1, 128], F32, kind="ExternalInput")
    MASKR = nc.dram_tensor("MASKR", [1, S], F32, kind="ExternalInput")
    DWR = nc.dram_tensor("DWR", [16, 8 * 128], F32, kind="ExternalInput")
    DBR = nc.dram_tensor("DBR", [1, 128], F32, kind="ExternalInput")

    ATTN = nc.dram_tensor("ATTN", [HPC, S, S], F32, kind="ExternalOutput")
    OUTP = nc.dram_tensor("OUTP", [HPC * 256, 128], F32, kind="ExternalOutput")

    with tile.TileContext(nc) as tc, contextlib.ExitStack() as ctx:
        const = ctx.enter_context(tc.tile_pool(name="const", bufs=1))
        stacks = ctx.enter_context(tc.tile_pool(name="stacks", bufs=1))

        wqx = const.tile([D, 128], F32)
        wqcq = const.tile([128, 1], F32)
        wqck = const.tile([128, 1], F32)
        wqrv = const.tile([1, 128], F32)
        maskr = const.tile([1, S], F32)
        dwr = const.tile([16, 8 * 128], F32)
        dbr = const.tile([1, 128], F32)
        nc.sync.dma_start(wqx[:], WQX[:])
        nc.sync.dma_start(wqcq[:], WQCOLQ[:])
        nc.sync.dma_start(wqck[:], WQCOLK[:])
        nc.sync.dma_start(wqrv[:], WQROWV[:])
        nc.sync.dma_start(maskr[:], MASKR[:])
        nc.sync.dma_start(dwr[:], DWR[:])
        nc.sync.dma_start(dbr[:], DBR[:])

        ident = const.tile([128, 128], F32)
        make_identity(nc, ident[:])
        zrow = const.tile([128, S], F32)
        nc.gpsimd.memset(zrow[:], 0.0)
        ones_row = const.tile([1, S], F32)
        nc.gpsimd.memset(ones_row[:], 1.0)
        ones4 = const.tile([128, 4], F32)
        nc.gpsimd.memset(ones4[:], 1.0)

        vhx = const.tile([128, NKT, HPC * 17], F32R)
        qxh, kxh = [], []
        for j in range(HPC):
            qx_t = stacks.tile([128, S], BF16, tag=f"qx{j}", name=f"qx{j}")
            kx_t = stacks.tile([128, S], BF16, tag=f"kx{j}", name=f"kx{j}")
            qxh.append(qx_t)
            kxh.append(kx_t)
        dsk = stacks.tile([128, 8 * 128], F32R)

        # psum pools - 8 banks: lt [128,1024]x2 = 4, ls [128,1024]x1 = 2,
        # ctxT [17,1024]/tp/po shared tag x1 = 2
        psA = ctx.enter_context(tc.tile_pool(name="psA", bufs=4, space="PSUM"))
        psC = ctx.enter_context(tc.tile_pool(name="psC", bufs=1, space="PSUM"))
        psS = ctx.enter_context(tc.tile_pool(name="psS", bufs=1, space="PSUM"))

        # ---- projection phase (scoped pool, freed afterwards) ----
        with tc.tile_pool(name="proj", bufs=1) as projp:
            qT = projp.tile([D, S], F32)
            kT = projp.tile([D, S], F32)
            vT = projp.tile([D, S], F32)
            tmp_q = projp.tile([128, S], F32)
            tmp_k = projp.tile([128, S], F32)
            nc.sync.dma_start(qT[:], QT[:])
            nc.sync.dma_start(kT[:], KT[:])
            nc.sync.dma_start(vT[:], VT[:])

            for chunk in range(4):
                sl = slice(chunk * 512, (chunk + 1) * 512)
                pq = psS.tile([128, 1024], F32, tag="ls")
                nc.tensor.matmul(pq[:, 0:512], wqx[:], qT[:, sl])
                nc.vector.tensor_scalar(out=tmp_q[:, sl], in0=pq[:, 0:512],
                                        scalar1=0.25, scalar2=wqcq[:],
                                        op0=Alu.mult, op1=Alu.add)
                pk = psS.tile([128, 1024], F32, tag="ls")
                nc.tensor.matmul(pk[:, 0:512], wqx[:], kT[:, sl])
                nc.vector.tensor_scalar(out=tmp_k[:, sl], in0=pk[:, 0:512],
                                        scalar1=1.0, scalar2=wqck[:],
                                        op0=Alu.mult, op1=Alu.add)

            for kt in range(NKT):
                sl = slice(kt * 128, (kt + 1) * 128)
                pv = psA.tile([128, 512], F32, tag="lt", name=f"pv{kt}")
                nc.tensor.matmul(pv[:, 0:128], vT[:, sl], wqx[:],
                                 start=True, stop=False)
                nc.tensor.matmul(pv[:, 0:128], ones_row[:, 0:128], wqrv[:],
                                 start=False, stop=True)
                vhx_v = vhx[:, kt, :].rearrange("p (h c) -> p h c", c=17)
                pv_v = pv[:, 0:128].rearrange("p (h c) -> p h c", c=32)
                nc.vector.tensor_copy(vhx_v[:, :, 0:16], pv_v[:, :, 0:16])
                nc.vector.tensor_copy(vhx_v[:, :, 16:17],
                                      ones4[:].rearrange("p (h c) -> p h c", c=1))

            hi_q = projp.tile([128, S], BF16)
            mid_q = projp.tile([128, S], BF16)
            hi_k = projp.tile([128, S], BF16)
            mid_k = projp.tile([128, S], BF16)
            nc.vector.tensor_copy(hi_q[:], tmp_q[:])
            nc.vector.tensor_sub(mid_q[:], tmp_q[:], hi_q[:])
            nc.vector.tensor_copy(hi_k[:], tmp_k[:])
            nc.vector.tensor_sub(mid_k[:], tmp_k[:], hi_k[:])
            for j in range(HPC):
                hs = slice(j * 32, j * 32 + 16)
                qx, kx = qxh[j], kxh[j]
                nc.vector.tensor_copy(qx[:], zrow[:])
                nc.vector.tensor_copy(qx[0:16], hi_q[hs])
                nc.vector.tensor_copy(qx[32:48], hi_q[hs])
                nc.vector.tensor_copy(qx[64:80], mid_q[hs])
                nc.vector.tensor_copy(qx[96:97], ones_row[:])
                nc.vector.tensor_copy(kx[:], zrow[:])
                nc.vector.tensor_copy(kx[0:16], hi_k[hs])
                nc.vector.tensor_copy(kx[32:48], mid_k[hs])
                nc.vector.tensor_copy(kx[64:80], hi_k[hs])
                nc.vector.tensor_copy(kx[96:97], maskr[:])

            nc.vector.tensor_copy(dsk[:], zrow[:, 0:1024])
            for s_lo in range(8):
                bsl = slice(s_lo * 128, (s_lo + 1) * 128)
                nc.vector.tensor_copy(dsk[0:16, bsl], dwr[:, bsl])
                nc.vector.tensor_sub(dsk[32:48, bsl], dwr[:, bsl],
                                     dsk[0:16, bsl].bitcast(F32))
                nc.vector.tensor_copy(dsk[64:80, bsl], dwr[:, bsl])
            nc.vector.tensor_copy(dsk[96:97, 0:128], dbr[:])

        etp = ctx.enter_context(tc.tile_pool(name="etp", bufs=16))
        esp = ctx.enter_context(tc.tile_pool(name="esp", bufs=2))
        attp = ctx.enter_context(tc.tile_pool(name="attp", bufs=2))
        ctxp = ctx.enter_context(tc.tile_pool(name="ctxp", bufs=2))
        small = ctx.enter_context(tc.tile_pool(name="small", bufs=2))

        for j in range(HPC):
            qx, kx = qxh[j], kxh[j]
            ctxTn = ctxp.tile([16, S], F32, tag="ctxTn")
            for qcc in range(2):
                qb = qcc * 1024
                ctxT = psC.tile([17, 1024], F32, tag="ctxT", name=f"ct{j}{qcc}")
                ets = []
                for kt in range(NKT):
                    ksl = slice(kt * 128, (kt + 1) * 128)
                    et = etp.tile([128, 1024], F32R, tag="et", name=f"et{kt}")
                    for eh in range(2):
                        lt = psA.tile([128, 512], F32, tag="lt")
                        nc.tensor.matmul(lt[:], kx[:, ksl],
                                         qx[:, qb + eh * 512:qb + (eh + 1) * 512])
                        nc.scalar.activation(et[:, eh * 512:(eh + 1) * 512],
                                             lt[:], Act.Exp)
                    ets.append(et)
                for kt in range(NKT):
                    nc.tensor.matmul(ctxT[:, 0:512],
                                     vhx[:, kt, j * 17:(j + 1) * 17],
                                     ets[kt][:, 0:512],
                                     start=(kt == 0), stop=(kt == NKT - 1))
                    nc.tensor.matmul(ctxT[:, 512:1024],
                                     vhx[:, kt, j * 17:(j + 1) * 17],
                                     ets[kt][:, 512:1024],
                                     start=(kt == 0), stop=(kt == NKT - 1))
                ctxT_sb = small.tile([17, 1024], F32, tag="ctxTsb")
                nc.vector.tensor_copy(ctxT_sb[:], ctxT[:])
                for qt in range(8):
                    q0 = qb + qt * 128
                    es = esp.tile([128, S], F32, tag="es")
                    for kh in range(2):
                        ls = psS.tile([128, 1024], F32, tag="ls")
                        for kc in range(2):
                            csl = slice(kh * 1024 + kc * 512,
                                        kh * 1024 + (kc + 1) * 512)
                            nc.tensor.matmul(ls[:, kc * 512:(kc + 1) * 512],
                                             qx[:, q0:q0 + 128],
                                             kx[:, csl])
                        nc.scalar.activation(es[:, kh * 1024:(kh + 1) * 1024],
                                             ls[:], Act.Exp)

                    tp = psC.tile([128, 17], F32, tag="ctxT",
                                  name=f"tp{j}{qcc}{qt}")
                    nc.tensor.transpose(tp[:],
                                        ctxT_sb[:, qt * 128:(qt + 1) * 128],
                                        ident[0:17, 0:17])
                    ctx_q = small.tile([128, 17], F32, tag="ctxq")
                    nc.vector.tensor_copy(ctx_q[:], tp[:])
                    recip = small.tile([128, 1], F32, tag="recip")
                    nc.vector.reciprocal(recip[:], ctx_q[:, 16:17])

                    attn_sb = attp.tile([128, S], F32, tag="attn")
                    nc.vector.tensor_scalar_mul(attn_sb[:], es[:], recip[:])
                    nc.sync.dma_start(ATTN[j, q0:q0 + 128, :], attn_sb[:])

                    ctx_n = small.tile([128, 16], F32, tag="ctxn")
                    nc.vector.tensor_scalar_mul(ctx_n[:], ctx_q[:, 0:16],
                                                recip[:])
                    tp2 = psC.tile([16, 128], F32, tag="ctxT",
                                   name=f"tq{j}{qcc}{qt}")
                    nc.tensor.transpose(tp2[:], ctx_n[:], ident[:])
                    nc.vector.tensor_copy(ctxTn[:, q0:q0 + 128], tp2[:])

            # out-projection for head j
            cstk = ctxp.tile([128, S], F32R, tag="cstk", bufs=1)
            nc.vector.tensor_copy(cstk[:], zrow[:])
            nc.vector.tensor_copy(cstk[0:16], ctxTn[:])
            nc.vector.tensor_copy(cstk[32:48], ctxTn[:])
            nc.vector.tensor_sub(cstk[64:80], ctxTn[:], cstk[0:16].bitcast(F32))
            nc.vector.tensor_copy(cstk[96:97], ones_row[:])
            for half in range(2):
                po = psC.tile([128, 128], F32, tag="ctxT", name=f"po{j}{half}")
                for s_lo in range(8):
                    base = half * 1024 + s_lo
                    lhsT = cstk[:, base:base + 8 * 127 + 1:8]
                    nc.tensor.matmul(po[:], lhsT,
                                     dsk[:, s_lo * 128:(s_lo + 1) * 128],
                                     start=(s_lo == 0), stop=(s_lo == 7))
                out_sb = small.tile([128, 128], F32, tag="outsb")
                nc.vector.tensor_copy(out_sb[:], po[:])
                nc.sync.dma_start(
                    OUTP[j * 256 + half * 128: j * 256 + (half + 1) * 128, :],
                    out_sb[:])

    nc.compile()
    return nc


def _get_nc():
    if 'nc' not in _BUILT:
        _BUILT['nc'] = _build_nc()
    return _BUILT['nc']


def kernel(q, k, v, mask, wq_w, wq_b, dense_w, dense_b):
    from concourse import bass_utils

    q = np.ascontiguousarray(q, dtype=np.float32)
    k = np.ascontiguousarray(k, dtype=np.float32)
    v = np.ascontiguousarray(v, dtype=np.float32)
    mask = np.asarray(mask, dtype=np.float32).reshape(B, S)
    wq_w = np.asarray(wq_w, dtype=np.float32)
    wq_b = np.asarray(wq_b, dtype=np.float32)
    dense_w = np.asarray(dense_w, dtype=np.float32)
    dense_b = np.asarray(dense_b, dtype=np.float32)

    in_maps = []
    for c in range(NCORES):
        b, hg = c // 2, c % 2
        wqxm = np.zeros((D, 128), dtype=np.float32)
        wqcq = np.zeros((128, 1), dtype=np.float32)
        wqck = np.zeros((128, 1), dtype=np.float32)
        wqrv = np.zeros((1, 128), dtype=np.float32)
        for j in range(HPC):
            hcols = slice((hg * HPC + j) * DP, (hg * HPC + j + 1) * DP)
            wqxm[:, j * 32:j * 32 + 16] = wq_w[:, hcols]
            wqcq[j * 32:j * 32 + 16, 0] = 0.25 * wq_b[hcols]
            wqck[j * 32:j * 32 + 16, 0] = wq_b[hcols]
            wqrv[0, j * 32:j * 32 + 16] = wq_b[hcols]
        in_maps.append({
            "QT": np.ascontiguousarray(q[b].T),
            "KT": np.ascontiguousarray(k[b].T),
            "VT": np.ascontiguousarray(v[b].T),
            "WQX": wqxm, "WQCOLQ": wqcq, "WQCOLK": wqck, "WQROWV": wqrv,
            "MASKR": (mask[b] * np.float32(-1e9)).reshape(1, S),
            "DWR": np.ascontiguousarray(
                dense_w.reshape(8, 16, 128).transpose(1, 0, 2).reshape(16, 8 * 128)),
            "DBR": dense_b.reshape(1, 128),
        })

    nc = _get_nc()
    results = bass_utils.run_bass_kernel_spmd(
        nc, in_maps, core_ids=list(range(NCORES))).results

    out = np.empty((B, S, D), dtype=np.float32)
    attn = np.empty((B, H, S, S), dtype=np.float32)
    for c in range(NCORES):
        b, hg = c // 2, c % 2
        attn[b, hg * HPC:(hg + 1) * HPC] = results[c]["ATTN"]
        out[b, hg * 1024:(hg + 1) * 1024] = results[c]["OUTP"]
    return out, attn


# revision 17
# speedup vs baseline: 1.0682x; 1.0682x over previous
"""MultiHeadAttention TRN2 kernel (B=4, S=2048, D=128, H=8) over 8 NeuronCores.

Sharding: core c handles batch b = c//2 and head-group hg = c%2 (4 heads).
Each core computes its 4 heads' attn [4, S, S] and its out rows [1024, 128].

Per-core algorithm:
  - projections qp/kp/vp on PE (fp32), heads padded to 32-partition bases
  - per-head stacked bf16 operand tiles qxh/kxh [128, S] (hi/mid split so a
    single K=128 matmul carries hi*hi + hi*mid + mid*hi ~ near-fp32 logits):
      qxh rows [0:16]=hi [32:48]=hi [64:80]=mid [96]=ones   (rest zero)
      kxh rows [0:16]=hi [32:48]=mid [64:80]=hi [96]=mask*-1e9
  - T-side (per 1024-wide q block): logitsT[k, q] = kx.T @ qx (bf16),
    expT = Exp(logitsT) in [128,1024] ACT ops (fp32r out),
    ctxT[17, 1024] += vhx.T @ expT (fp32r; col 16 of vhx = ones -> row sums)
  - straight side (per q tile): logits [q,k] = qx.T @ kx (mask via row 96),
    exp (ACT) -> attn = exp * recip[q] (DVE) -> DMA out
  - ctx sandwich: ctxT -> PE transpose -> recip -> normalize -> transpose back
  - out-proj: reshape-aware K=128 stacked fp32r matmul with dense_w hi/lo
"""
import sys

if '/opt/trn_rl_repo' not in sys.path:
    sys.path.insert(0, '/opt/trn_rl_repo')

import contextlib

import numpy as np

B, S, D, H = 4, 2048, 128, 8
DP = D // H
HPC = 4
NCORES = 8
NKT = S // 128
QC = 512

_BUILT = {}


def _build_nc():
    import concourse.mybir as mybir
    import concourse.tile as tile
    from concourse import bacc
    from concourse.masks import make_identity

    F32 = mybir.dt.float32
    F32R = mybir.dt.float32r
    BF16 = mybir.dt.bfloat16
    Act = mybir.ActivationFunctionType
    Alu = mybir.AluOpType

    nc = bacc.Bacc("TRN2", target_bir_lowering=False, debug=False)

    QT = nc.dram_tensor("QT", [D, S], F32, kind="ExternalInput")
    KT = nc.dram_tensor("KT", [D, S], F32, kind="ExternalInput")
    VT = nc.dram_tensor("VT", [D, S], F32, kind="ExternalInput")
    WQX = nc.dram_tensor("WQX", [D, 128], F32, kind="ExternalInput")
    WQCOLQ = nc.dram_tensor("WQCOLQ", [128, 1], F32, kind="ExternalInput")
    WQCOLK = nc.dram_tensor("WQCOLK", [128, 1], F32, kind="ExternalInput")
    WQROWV = nc.dram_tensor("WQROWV", [1, 128], F32, kind="ExternalInput")
    MASKR = nc.dram_tensor("MASKR", [1, S], F32, kind="ExternalInput")
    DWR = nc.dram_tensor("DWR", [16, 8 * 128], F32, kind="ExternalInput")
    DBR = nc.dram_tensor("DBR", [1, 128], F32, kind="ExternalInput")

    ATTN = nc.dram_tensor("ATTN", [HPC, S, S], F32, kind="ExternalOutput")
    OUTP = nc.dram_tensor("OUTP", [HPC * 256, 128], F32, kind="ExternalOutput")

    with tile.TileContext(nc) as tc, contextlib.ExitStack() as ctx:
        const = ctx.enter_context(tc.tile_pool(name="const", bufs=1))
        stacks = ctx.enter_context(tc.tile_pool(name="stacks", bufs=1))

        wqx = const.tile([D, 128], F32)
        wqcq = const.tile([128, 1], F32)
        wqck = const.tile([128, 1], F32)
        wqrv = const.tile([1, 128], F32)
        maskr = const.tile([1, S], F32)
        dwr = const.tile([16, 8 * 128], F32)
        dbr = const.tile([1, 128], F32)
        nc.sync.dma_start(wqx[:], WQX[:])
        nc.sync.dma_start(wqcq[:], WQCOLQ[:])
        nc.sync.dma_start(wqck[:], WQCOLK[:])
        nc.sync.dma_start(wqrv[:], WQROWV[:])
        nc.sync.dma_start(maskr[:], MASKR[:])
        nc.sync.dma_start(dwr[:], DWR[:])
        nc.sync.dma_start(dbr[:], DBR[:])

        ident = const.tile([128, 128], F32)
        make_identity(nc, ident[:])
        zrow = const.tile([128, S], F32)
        nc.gpsimd.memset(zrow[:], 0.0)
        ones_row = const.tile([1, S], F32)
        nc.gpsimd.memset(ones_row[:], 1.0)
        ones4 = const.tile([128, 4], F32)
        nc.gpsimd.memset(ones4[:], 1.0)

        vhx = const.tile([128, NKT, HPC * 17], F32R)
        qxh, kxh = [], []
        for j in range(HPC):
            qx_t = stacks.tile([128, S], BF16, tag=f"qx{j}", name=f"qx{j}")
            kx_t = stacks.tile([128, S], BF16, tag=f"kx{j}", name=f"kx{j}")
            qxh.append(qx_t)
            kxh.append(kx_t)
        dsk = stacks.tile([128, 8 * 128], F32R)

        # psum pools - 8 banks: lt [128,1024]x2 = 4, ls [128,1024]x1 = 2,
        # ctxT [17,1024]/tp/po shared tag x1 = 2
        psA = ctx.enter_context(tc.tile_pool(name="psA", bufs=3, space="PSUM"))
        psC = ctx.enter_context(tc.tile_pool(name="psC", bufs=1, space="PSUM"))

        # ---- projection phase (scoped pool, freed afterwards) ----
        with tc.tile_pool(name="proj", bufs=1) as projp:
            qT = projp.tile([D, S], F32)
            kT = projp.tile([D, S], F32)
            vT = projp.tile([D, S], F32)
            tmp_q = projp.tile([128, S], F32)
            tmp_k = projp.tile([128, S], F32)
            nc.sync.dma_start(qT[:], QT[:])
            nc.sync.dma_start(kT[:], KT[:])
            nc.sync.dma_start(vT[:], VT[:])

            for chunk in range(4):
                sl = slice(chunk * 512, (chunk + 1) * 512)
                pq = psA.tile([128, 1024], F32, tag="lt", name=f"pq{chunk}")
                nc.tensor.matmul(pq[:, 0:512], wqx[:], qT[:, sl])
                nc.vector.tensor_scalar(out=tmp_q[:, sl], in0=pq[:, 0:512],
                                        scalar1=0.25, scalar2=wqcq[:],
                                        op0=Alu.mult, op1=Alu.add)
                pk = psA.tile([128, 1024], F32, tag="lt", name=f"pk{chunk}")
                nc.tensor.matmul(pk[:, 0:512], wqx[:], kT[:, sl])
                nc.vector.tensor_scalar(out=tmp_k[:, sl], in0=pk[:, 0:512],
                                        scalar1=1.0, scalar2=wqck[:],
                                        op0=Alu.mult, op1=Alu.add)

            for kt in range(NKT):
                sl = slice(kt * 128, (kt + 1) * 128)
                pv = psA.tile([128, 1024], F32, tag="lt", name=f"pv{kt}")
                nc.tensor.matmul(pv[:, 0:128], vT[:, sl], wqx[:],
                                 start=True, stop=False)
                nc.tensor.matmul(pv[:, 0:128], ones_row[:, 0:128], wqrv[:],
                                 start=False, stop=True)
                vhx_v = vhx[:, kt, :].rearrange("p (h c) -> p h c", c=17)
                pv_v = pv[:, 0:128].rearrange("p (h c) -> p h c", c=32)
                nc.vector.tensor_copy(vhx_v[:, :, 0:16], pv_v[:, :, 0:16])
                nc.vector.tensor_copy(vhx_v[:, :, 16:17],
                                      ones4[:].rearrange("p (h c) -> p h c", c=1))

            hi_q = projp.tile([128, S], BF16)
            mid_q = projp.tile([128, S], BF16)
            hi_k = projp.tile([128, S], BF16)
            mid_k = projp.tile([128, S], BF16)
            nc.vector.tensor_copy(hi_q[:], tmp_q[:])
            nc.vector.tensor_sub(mid_q[:], tmp_q[:], hi_q[:])
            nc.vector.tensor_copy(hi_k[:], tmp_k[:])
            nc.vector.tensor_sub(mid_k[:], tmp_k[:], hi_k[:])
            for j in range(HPC):
                hs = slice(j * 32, j * 32 + 16)
                qx, kx = qxh[j], kxh[j]
                nc.vector.tensor_copy(qx[:], zrow[:])
                nc.vector.tensor_copy(qx[0:16], hi_q[hs])
                nc.vector.tensor_copy(qx[32:48], hi_q[hs])
                nc.vector.tensor_copy(qx[64:80], mid_q[hs])
                nc.vector.tensor_copy(qx[96:97], ones_row[:])
                nc.vector.tensor_copy(kx[:], zrow[:])
                nc.vector.tensor_copy(kx[0:16], hi_k[hs])
                nc.vector.tensor_copy(kx[32:48], mid_k[hs])
                nc.vector.tensor_copy(kx[64:80], hi_k[hs])
                nc.vector.tensor_copy(kx[96:97], maskr[:])

            nc.vector.tensor_copy(dsk[:], zrow[:, 0:1024])
            for s_lo in range(8):
                bsl = slice(s_lo * 128, (s_lo + 1) * 128)
                nc.vector.tensor_copy(dsk[0:16, bsl], dwr[:, bsl])
                nc.vector.tensor_sub(dsk[32:48, bsl], dwr[:, bsl],
                                     dsk[0:16, bsl].bitcast(F32))
                nc.vector.tensor_copy(dsk[64:80, bsl], dwr[:, bsl])
            nc.vector.tensor_copy(dsk[96:97, 0:128], dbr[:])

        etp = ctx.enter_context(tc.tile_pool(name="etp", bufs=16))
        esp = ctx.enter_context(tc.tile_pool(name="esp", bufs=2))
        attp = ctx.enter_context(tc.tile_pool(name="attp", bufs=2))
        ctxp = ctx.enter_context(tc.tile_pool(name="ctxp", bufs=2))
        small = ctx.enter_context(tc.tile_pool(name="small", bufs=2))

        for j in range(HPC):
            qx, kx = qxh[j], kxh[j]
            ctxTn = ctxp.tile([16, S], F32, tag="ctxTn")
            for qcc in range(2):
                qb = qcc * 1024
                ctxT = psC.tile([17, 1024], F32, tag="ctxT", name=f"ct{j}{qcc}")
                ets = []
                for kt in range(NKT):
                    ksl = slice(kt * 128, (kt + 1) * 128)
                    lt = psA.tile([128, 1024], F32, tag="lt")
                    nc.tensor.matmul(lt[:, 0:512], kx[:, ksl],
                                     qx[:, qb:qb + 512])
                    nc.tensor.matmul(lt[:, 512:1024], kx[:, ksl],
                                     qx[:, qb + 512:qb + 1024])
                    et = etp.tile([128, 1024], F32R, tag="et", name=f"et{kt}")
                    nc.scalar.activation(et[:], lt[:], Act.Exp)
                    ets.append(et)
                for kt in range(NKT):
                    nc.tensor.matmul(ctxT[:, 0:512],
                                     vhx[:, kt, j * 17:(j + 1) * 17],
                                     ets[kt][:, 0:512],
                                     start=(kt == 0), stop=(kt == NKT - 1))
                    nc.tensor.matmul(ctxT[:, 512:1024],
                                     vhx[:, kt, j * 17:(j + 1) * 17],
                                     ets[kt][:, 512:1024],
                                     start=(kt == 0), stop=(kt == NKT - 1))
                ctxT_sb = small.tile([17, 1024], F32, tag="ctxTsb")
                nc.vector.tensor_copy(ctxT_sb[:], ctxT[:])
                for qt in range(8):
                    q0 = qb + qt * 128
                    es = esp.tile([128, S], F32, tag="es")
                    for kh in range(2):
                        ls = psA.tile([128, 1024], F32, tag="lt", name=f"ls{j}{qcc}{qt}{kh}")
                        for kc in range(2):
                            csl = slice(kh * 1024 + kc * 512,
                                        kh * 1024 + (kc + 1) * 512)
                            nc.tensor.matmul(ls[:, kc * 512:(kc + 1) * 512],
                                             qx[:, q0:q0 + 128],
                                             kx[:, csl])
                        nc.scalar.activation(es[:, kh * 1024:(kh + 1) * 1024],
                                             ls[:], Act.Exp)

                    tp = psC.tile([128, 17], F32, tag="ctxT",
                                  name=f"tp{j}{qcc}{qt}")
                    nc.tensor.transpose(tp[:],
                                        ctxT_sb[:, qt * 128:(qt + 1) * 128],
                                        ident[0:17, 0:17])
                    ctx_q = small.tile([128, 17], F32, tag="ctxq")
                    nc.vector.tensor_copy(ctx_q[:], tp[:])
                    recip = small.tile([128, 1], F32, tag="recip")
                    nc.vector.reciprocal(recip[:], ctx_q[:, 16:17])

                    attn_sb = attp.tile([128, S], F32, tag="attn")
                    nc.vector.tensor_scalar_mul(attn_sb[:], es[:], recip[:])
                    nc.sync.dma_start(ATTN[j, q0:q0 + 128, :], attn_sb[:])

                    ctx_n = small.tile([128, 16], F32, tag="ctxn")
                    nc.vector.tensor_scalar_mul(ctx_n[:], ctx_q[:, 0:16],
                                                recip[:])
                    tp2 = psC.tile([16, 128], F32, tag="ctxT",
                                   name=f"tq{j}{qcc}{qt}")
                    nc.tensor.transpose(tp2[:], ctx_n[:], ident[:])
                    nc.vector.tensor_copy(ctxTn[:, q0:q0 + 128], tp2[:])

            # out-projection for head j
            cstk = ctxp.tile([128, S], F32R, tag="cstk", bufs=1)
            nc.vector.tensor_copy(cstk[:], zrow[:])
            nc.vector.tensor_copy(cstk[0:16], ctxTn[:])
            nc.vector.tensor_copy(cstk[32:48], ctxTn[:])
            nc.vector.tensor_sub(cstk[64:80], ctxTn[:], cstk[0:16].bitcast(F32))
            nc.vector.tensor_copy(cstk[96:97], ones_row[:])
            for half in range(2):
                po = psC.tile([128, 128], F32, tag="ctxT", name=f"po{j}{half}")
                for s_lo in range(8):
                    base = half * 1024 + s_lo
                    lhsT = cstk[:, base:base + 8 * 127 + 1:8]
                    nc.tensor.matmul(po[:], lhsT,
                                     dsk[:, s_lo * 128:(s_lo + 1) * 128],
                                     start=(s_lo == 0), stop=(s_lo == 7))
                out_sb = small.tile([128, 128], F32, tag="outsb")
                nc.vector.tensor_copy(out_sb[:], po[:])
                nc.sync.dma_start(
                    OUTP[j * 256 + half * 128: j * 256 + (half + 1) * 128, :],
                    out_sb[:])

    nc.compile()
    return nc


def _get_nc():
    if 'nc' not in _BUILT:
        _BUILT['nc'] = _build_nc()
    return _BUILT['nc']


def kernel(q, k, v, mask, wq_w, wq_b, dense_w, dense_b):
    from concourse import bass_utils

    q = np.ascontiguousarray(q, dtype=np.float32)
    k = np.ascontiguousarray(k, dtype=np.float32)
    v = np.ascontiguousarray(v, dtype=np.float32)
    mask = np.asarray(mask, dtype=np.float32).reshape(B, S)
    wq_w = np.asarray(wq_w, dtype=np.float32)
    wq_b = np.asarray(wq_b, dtype=np.float32)
    dense_w = np.asarray(dense_w, dtype=np.float32)
    dense_b = np.asarray(dense_b, dtype=np.float32)

    in_maps = []
    for c in range(NCORES):
        b, hg = c // 2, c % 2
        wqxm = np.zeros((D, 128), dtype=np.float32)
        wqcq = np.zeros((128, 1), dtype=np.float32)
        wqck = np.zeros((128, 1), dtype=np.float32)
        wqrv = np.zeros((1, 128), dtype=np.float32)
        for j in range(HPC):
            hcols = slice((hg * HPC + j) * DP, (hg * HPC + j + 1) * DP)
            wqxm[:, j * 32:j * 32 + 16] = wq_w[:, hcols]
            wqcq[j * 32:j * 32 + 16, 0] = 0.25 * wq_b[hcols]
            wqck[j * 32:j * 32 + 16, 0] = wq_b[hcols]
            wqrv[0, j * 32:j * 32 + 16] = wq_b[hcols]
        in_maps.append({
            "QT": np.ascontiguousarray(q[b].T),
            "KT": np.ascontiguousarray(k[b].T),
            "VT": np.ascontiguousarray(v[b].T),
            "WQX": wqxm, "WQCOLQ": wqcq, "WQCOLK": wqck, "WQROWV": wqrv,
            "MASKR": (mask[b] * np.float32(-1e9)).reshape(1, S),
            "DWR": np.ascontiguousarray(
                dense_w.reshape(8, 16, 128).transpose(1, 0, 2).reshape(16, 8 * 128)),
            "DBR": dense_b.reshape(1, 128),
        })

    nc = _get_nc()
    results = bass_utils.run_bass_kernel_spmd(
        nc, in_maps, core_ids=list(range(NCORES))).results

    out = np.empty((B, S, D), dtype=np.float32)
    attn = np.empty((B, H, S, S), dtype=np.float32)
    for c in range(NCORES):
        b, hg = c // 2, c % 2
        attn[b, hg * HPC:(hg + 1) * HPC] = results[c]["ATTN"]
        out[b, hg * 1024:(hg + 1) * 1024] = results[c]["OUTP"]
    return out, attn
